# revision 14
# baseline (speedup 1.0000x reference)
"""ANFIS Trainium2 kernel (8 NeuronCores, Bass/Tile).

Math (reference):
  mfs[b,i,j] = exp(-(x[b,i]-centers[i,j])^2 / (2*widths[i,j]^2))   [1024,8,4]
  w[b,r]     = prod_i mfs[b,i,idx_i(r)]    r in [0, 4^8=65536), i0 slowest
  w        <- w / sum_r w
  out[b,n]   = sum_r w[b,r] * ([x[b],1] . rule_params[r,:,n])      [1024,16]

Key structure: w = wA (x) wB with wA over dims 0..2 (64 vals), wB over dims
3..7 (1024 vals); r = rA*1024 + rB.  The denominator factorizes:
sum_r w = prod_i (sum_j mfs[b,i,j]).

Sharding: rA split across 8 cores (8 local rA each = contiguous 8192-rule
row blocks of rule_params).  Per core:
  out_c[b,n] = sum_{rA local} wA[b,rA]/denom[b] *
               sum_i xb[b,i] (sum_rB wB[b,rB] rp[rA*1024+rB, i*16+n])
then a 2-way-split ReduceScatter(add) over cores; each core emits its two
64-row output shards, host reassembles.

The inner sum_rB is a matmul (contraction over rB on partitions), so wB^T
is needed in [rB, b] layout.  wB is built in [b, rB] layout with strided
free-axis outer products over dims 3..6 (w3456, 256 wide per b-tile),
scaled by the dim-7 membership (4 variants), and transposed 128x128 via
the DMA xbar transpose engine (bf16) -- no TensorE/PSUM involvement.
rB is enumerated as rB' = i7*256 + q (q = dims 3..6); rule_params rows are
permuted on the host to match.  Matmul operands are bf16 (fp32 matmuls
cost two PE passes); PSUM accumulation stays fp32.
"""

import sys

sys.path.insert(0, "/opt/trn_rl_repo")

import numpy as np

import concourse.bass as bass
import concourse.bacc as bacc
import concourse.tile as tile
import concourse.mybir as mybir
from concourse.ap import AP
from concourse.bass_utils import run_bass_kernel_spmd

F32 = mybir.dt.float32
BF16 = mybir.dt.bfloat16
MULT = mybir.AluOpType.mult
ADD = mybir.AluOpType.add
SUB = mybir.AluOpType.subtract
EXP = mybir.ActivationFunctionType.Exp
AXX = mybir.AxisListType.X

N_CORES = 8
B = 1024
BT = 8          # batch tiles of 128
D = 8           # input dims
DX = D + 1      # xb width (x plus ones column)
M = 4           # membership fns per dim
NO = 16         # outputs
C = DX * NO                 # 144
NRA = 64        # 4^3 (dims 0..2)
RA_LOC = NRA // N_CORES     # 8 local rA per core
NRB = 1024      # 4^5 (dims 3..7)
KT = 8          # rB partition tiles of 128
# rA groups per psum bank (N<=512): {0,1,2},{3,4,5},{6,7}
GROUPS = [(0, 3), (3, 3), (6, 2)]
SC = RA_LOC * C  # 1152


def _v(t, off, dims):
    """Custom free-dim view of a [128, F] SBUF tile AP.

    t: AP covering the full tile ([128, F]); off: element offset within the
    partition row; dims: list of (step, count) free dims, outer..inner.
    """
    part = list(t.ap[0])
    return AP(
        tensor=t.tensor,
        offset=t.offset + off,
        ap=[part] + [[s, n] for (s, n) in dims],
    )


def build_nc():
    nc = bacc.Bacc("TRN2", target_bir_lowering=False, debug=False,
                   num_devices=N_CORES)

    NSM = BT * DX + 2 * D * M + 2 * RA_LOC * 3  # 184
    small_d = nc.declare_dram_parameter("small", [128, NSM], F32, isOutput=False)
    eye_d = nc.declare_dram_parameter("eye", [128, 128], BF16, isOutput=False)
    rp_d = nc.declare_dram_parameter("rp", [128, KT * SC], BF16, isOutput=False)
    out_d = nc.declare_dram_parameter("out", [128, NO], BF16, isOutput=True)

    with tile.TileContext(nc) as tc:
        with (
            tc.tile_pool(name="const", bufs=1) as cpool,
            tc.tile_pool(name="rp", bufs=1) as rppool,
            tc.tile_pool(name="wbt", bufs=1) as wbtpool,
            tc.tile_pool(name="work", bufs=2) as work,
            tc.tile_pool(name="dj", bufs=6) as djpool,
            tc.tile_pool(name="psD", bufs=1, space="PSUM") as psDp,
            tc.tile_pool(name="evac", bufs=2) as evpool,
            tc.tile_pool(name="ps0", bufs=2, space="PSUM") as ps0p,
            tc.tile_pool(name="ps1", bufs=2, space="PSUM") as ps1p,
            tc.tile_pool(name="ps2", bufs=2, space="PSUM") as ps2p,
            tc.tile_pool(name="dram", bufs=1, space="DRAM") as dram,
        ):
            # ---- input DMAs (one small slab + eye + rp slabs) ----
            small = cpool.tile([128, NSM], F32, tag="small")
            eye = cpool.tile([128, 128], BF16, tag="eye")
            rp = rppool.tile([128, KT * SC], BF16, tag="rp")

            nc.sync.dma_start(small[:], small_d[:])
            nc.sync.dma_start(eye[:], eye_d[:])
            xab = small[:, 0: BT * DX]
            cb = small[:, BT * DX: BT * DX + D * M]
            wt = small[:, BT * DX + D * M: BT * DX + 2 * D * M]
            cA = small[:, BT * DX + 2 * D * M: BT * DX + 2 * D * M + RA_LOC * 3]
            wtA = small[:, BT * DX + 2 * D * M + RA_LOC * 3: NSM]
            for kt in range(KT):
                nc.sync.dma_start(rp[:, kt * SC:(kt + 1) * SC],
                                  rp_d[:, kt * SC:(kt + 1) * SC])

            # ---- membership values mfs [128, (bt, i, j)] ----
            t32a = work.tile([128, D * M], F32, tag="t32")
            t32b = work.tile([128, D * M], F32, tag="t32")
            nw = cpool.tile([128, D * M], F32, tag="nw")
            nc.vector.tensor_tensor(t32a[:], wt, wt, op=MULT)
            nc.vector.tensor_scalar_mul(t32b[:], t32a[:], -2.0)
            nc.vector.reciprocal(nw[:], t32b[:])

            MF = BT * D * M  # 256
            dif = work.tile([128, MF], F32, tag="dif")
            nc.vector.tensor_tensor(
                dif[:],
                _v(xab, 0, [(DX, BT), (1, D), (0, M)]),
                _v(cb, 0, [(0, BT), (1, D * M)]),
                op=SUB,
            )
            d2 = work.tile([128, MF], F32, tag="d2")
            nc.vector.tensor_tensor(d2[:], dif[:], dif[:], op=MULT)
            d2s = work.tile([128, MF], F32, tag="d2s")
            nc.vector.tensor_tensor(
                d2s[:], d2[:], _v(nw[:], 0, [(0, BT), (1, D * M)]), op=MULT)
            mfs = cpool.tile([128, MF], F32, tag="mfs")
            nc.scalar.activation(mfs[:], d2s[:], EXP)

            # ---- wB factors over dims 3..6, [b, (bt, q)] layout, bf16 ----
            w34 = work.tile([128, BT * 16], BF16, tag="w34")
            nc.vector.tensor_tensor(
                w34[:],
                _v(mfs[:], 3 * M, [(D * M, BT), (1, M), (0, M)]),
                _v(mfs[:], 4 * M, [(D * M, BT), (0, M), (1, M)]),
                op=MULT)
            w56 = work.tile([128, BT * 16], BF16, tag="w56")
            nc.vector.tensor_tensor(
                w56[:],
                _v(mfs[:], 5 * M, [(D * M, BT), (1, M), (0, M)]),
                _v(mfs[:], 6 * M, [(D * M, BT), (0, M), (1, M)]),
                op=MULT)
            w3456 = cpool.tile([128, BT * 256], BF16, tag="w3456")
            for bt in range(BT):
                i_w3456 = nc.vector.tensor_tensor(
                    w3456[:, bt * 256:(bt + 1) * 256],
                    _v(w34[:], bt * 16, [(1, 16), (0, 16)]),
                    _v(w56[:], bt * 16, [(0, 16), (1, 16)]),
                    op=MULT)

            # ---- wB^T: pre-scale by dim-7 membership, PE-transpose vs eye ----
            # wbt[p, kt*1024 + b] = wB'[rB' = kt*128+p, b],  rB' = i7*256+q
            # w3s = w3456 * mfs7_j;  psum = w3s.T @ I
            wbt = wbtpool.tile([128, KT * B], BF16, tag="wbt")
            for bt in range(BT):
                w3s = []
                for j in range(M):
                    w3sj = djpool.tile([128, 256], BF16, tag="w3s", name="w3s")
                    nc.vector.tensor_scalar_mul(
                        w3sj[:], w3456[:, bt * 256:(bt + 1) * 256],
                        mfs[:, bt * D * M + 7 * M + j:
                            bt * D * M + 7 * M + j + 1])
                    w3s.append(w3sj)
                psD = [psDp.tile([128, 512], F32, tag="psD0", name="psD0"),
                       psDp.tile([128, 512], F32, tag="psD1", name="psD1")]
                for j in range(M):
                    for qh in range(2):
                        kt = 2 * j + qh
                        m, t = kt // 4, kt % 4
                        nc.tensor.matmul(
                            psD[m][:, t * 128:(t + 1) * 128],
                            w3s[j][:, qh * 128:(qh + 1) * 128], eye[:],
                            start=True, stop=True)
                for m in range(2):
                    nc.scalar.copy(
                        _v(wbt[:], (4 * m) * B + bt * 128, [(B, 4), (1, 128)]),
                        psD[m][:])

            # ---- local wA [128, (bt, r)] from per-core selected centers ----
            t24a = work.tile([128, RA_LOC * 3], F32, tag="t24")
            t24b = work.tile([128, RA_LOC * 3], F32, tag="t24")
            nwA = cpool.tile([128, RA_LOC * 3], F32, tag="nwA")
            i_t24a = nc.vector.tensor_tensor(t24a[:], wtA, wtA, op=MULT)
            tile.add_dep_helper(i_t24a.ins, i_w3456.ins, sync=False,
                                reason="keep wbt build chain ahead of wA chain")
            nc.vector.tensor_scalar_mul(t24b[:], t24a[:], -2.0)
            nc.vector.reciprocal(nwA[:], t24b[:])

            NA = BT * RA_LOC * 3  # 192
            dA = work.tile([128, NA], F32, tag="dA")
            nc.vector.tensor_tensor(
                dA[:],
                _v(xab, 0, [(DX, BT), (0, RA_LOC), (1, 3)]),
                _v(cA, 0, [(0, BT), (3, RA_LOC), (1, 3)]),
                op=SUB,
            )
            dA2 = work.tile([128, NA], F32, tag="dA2")
            nc.vector.tensor_tensor(dA2[:], dA[:], dA[:], op=MULT)
            dA2s = work.tile([128, NA], F32, tag="dA2s")
            nc.vector.tensor_tensor(
                dA2s[:], dA2[:],
                _v(nwA[:], 0, [(0, BT), (3, RA_LOC), (1, 3)]), op=MULT)
            eA = work.tile([128, BT * RA_LOC], F32, tag="eA")
            nc.vector.reduce_sum(
                eA[:], _v(dA2s[:], 0, [(3 * RA_LOC, BT), (3, RA_LOC), (1, 3)]),
                axis=AXX)
            wA = cpool.tile([128, BT * RA_LOC], F32, tag="wA")
            nc.scalar.activation(wA[:], eA[:], EXP)

            # ---- denominator: denom[b] = prod_i sum_j mfs ----
            s = work.tile([128, BT * D], F32, tag="s")
            nc.vector.reduce_sum(
                s[:], _v(mfs[:], 0, [(M, BT * D), (1, M)]), axis=AXX)
            p1 = work.tile([128, BT * 4], F32, tag="p1")
            nc.vector.tensor_tensor(
                p1[:], _v(s[:], 0, [(D, BT), (1, 4)]),
                _v(s[:], 4, [(D, BT), (1, 4)]), op=MULT)
            p2 = work.tile([128, BT * 2], F32, tag="p2")
            nc.vector.tensor_tensor(
                p2[:], _v(p1[:], 0, [(4, BT), (1, 2)]),
                _v(p1[:], 2, [(4, BT), (1, 2)]), op=MULT)
            p3 = work.tile([128, BT], F32, tag="p3")
            nc.vector.tensor_tensor(
                p3[:], _v(p2[:], 0, [(2, BT)]), _v(p2[:], 1, [(2, BT)]),
                op=MULT)
            invd = cpool.tile([128, BT], F32, tag="invd")
            nc.vector.reciprocal(invd[:], p3[:])

            wAn = cpool.tile([128, BT * RA_LOC], F32, tag="wAn")
            for bt in range(BT):
                nc.vector.tensor_scalar_mul(
                    wAn[:, bt * RA_LOC:(bt + 1) * RA_LOC],
                    wA[:, bt * RA_LOC:(bt + 1) * RA_LOC],
                    invd[:, bt:bt + 1])

            # ---- main matmuls + evac ----
            partial = dram.tile([B, NO], BF16)
            for bt in range(BT):
                ps = [ps0p.tile([128, GROUPS[0][1] * C], F32, tag="ps0", name="ps0"),
                      ps1p.tile([128, GROUPS[1][1] * C], F32, tag="ps1", name="ps1"),
                      ps2p.tile([128, GROUPS[2][1] * C], F32, tag="ps2", name="ps2")]
                for kt in range(KT):
                    lhsT = wbt[:, kt * B + bt * 128: kt * B + (bt + 1) * 128]
                    for g, (r0, nr) in enumerate(GROUPS):
                        nc.tensor.matmul(
                            ps[g][:], lhsT,
                            _v(rp[:], (kt * RA_LOC + r0) * C, [(C, nr), (1, C)]),
                            start=(kt == 0), stop=(kt == KT - 1))
                # xsc[b, r*144 + i*16 + n] = psum * wAn[b,r] * xb[b,i]
                # via G[b, r*9+i] = wAn[b,r]*xb[b,i], then one big reduce
                G = evpool.tile([128, RA_LOC * DX], F32, tag="G")
                nc.vector.tensor_tensor(
                    G[:],
                    _v(wAn[:], bt * RA_LOC, [(1, RA_LOC), (0, DX)]),
                    _v(xab, bt * DX, [(0, RA_LOC), (1, DX)]), op=MULT)
                xsc = evpool.tile([128, SC], F32, tag="xsc")
                for g, (r0, nr) in enumerate(GROUPS):
                    nc.vector.tensor_tensor(
                        xsc[:, r0 * C:(r0 + nr) * C], ps[g][:],
                        _v(G[:], r0 * DX, [(DX, nr), (1, DX), (0, NO)]),
                        op=MULT)
                ob = evpool.tile([128, NO], F32, tag="ob")
                nc.vector.reduce_sum(
                    ob[:], _v(xsc[:], 0, [(1, NO), (NO, RA_LOC * DX)]),
                    axis=AXX)
                obh = evpool.tile([128, NO], BF16, tag="obh")
                nc.vector.tensor_copy(obh[:], ob[:])
                nc.sync.dma_start(partial[bt * 128:(bt + 1) * 128, :], obh[:])

            # ---- reduce-scatter (bf16); each core keeps its 128-row shard ----
            rs = dram.tile([B // N_CORES, NO], BF16)
            nc.gpsimd.collective_compute(
                "ReduceScatter", ADD,
                replica_groups=[list(range(N_CORES))],
                ins=[partial.opt()], outs=[rs.opt()])
            nc.sync.dma_start(out_d[:], rs[:])

    nc.compile()
    return nc


_NC_CACHE = None


def _get_nc():
    global _NC_CACHE
    if _NC_CACHE is None:
        _NC_CACHE = build_nc()
    return _NC_CACHE


def _prep_in_maps(x, centers, widths, rule_params):
    import ml_dtypes

    x = np.asarray(x, np.float32)
    centers = np.asarray(centers, np.float32)
    widths = np.asarray(widths, np.float32)
    rule_params = np.asarray(rule_params, np.float32)

    # xab[p, bt*9+i] = x[bt*128+p, i] for i<8; 1.0 at i=8
    xab = np.ones((128, BT, DX), np.float32)
    xab[:, :, :D] = x.reshape(BT, 128, D).transpose(1, 0, 2)
    xab = xab.reshape(128, BT * DX)
    cb = np.broadcast_to(centers.reshape(1, D * M), (128, D * M))
    wt = np.broadcast_to(widths.reshape(1, D * M), (128, D * M))
    eye = np.eye(128, dtype=ml_dtypes.bfloat16)

    # rule_params rows r = rA*1024 + q*4 + j -> per core [p, kt, rA, c]
    # with row order rB' = j*256 + q, kt = rB' tile of 128.
    rp4 = rule_params.reshape(NRA, 256, M, C).transpose(0, 2, 1, 3)
    rp4 = rp4.reshape(NRA, NRB, C)

    in_maps = []
    for c in range(N_CORES):
        ra0 = c * RA_LOC
        idx = np.empty((RA_LOC, 3), np.int64)
        for r in range(RA_LOC):
            ra = ra0 + r
            idx[r] = [(ra >> 4) & 3, (ra >> 2) & 3, ra & 3]
        k = np.arange(3)
        cA = centers[k[None, :], idx]
        wtA = widths[k[None, :], idx]
        cA = np.broadcast_to(cA.reshape(1, RA_LOC * 3), (128, RA_LOC * 3))
        wtA = np.broadcast_to(wtA.reshape(1, RA_LOC * 3), (128, RA_LOC * 3))
        small = np.ascontiguousarray(
            np.concatenate([xab, cb, wt, cA, wtA], axis=1, dtype=np.float32))

        rp_c = rp4[ra0:ra0 + RA_LOC]                     # [8, 1024, 144]
        rp_c = rp_c.reshape(RA_LOC, KT, 128, C).transpose(2, 1, 0, 3)
        rp_c = np.ascontiguousarray(
            rp_c.reshape(128, KT * SC)).astype(ml_dtypes.bfloat16)

        in_maps.append({"small": small, "eye": eye, "rp": rp_c})
    return in_maps


def kernel(x, centers, widths, rule_params, _trace=False):
    nc = _get_nc()
    in_maps = _prep_in_maps(x, centers, widths, rule_params)
    res = run_bass_kernel_spmd(nc, in_maps, core_ids=list(range(N_CORES)),
                               trace=_trace)
    out = np.concatenate(
        [res.results[c]["out"].astype(np.float32) for c in range(N_CORES)],
        axis=0)
    if _trace:
        kernel._last_exec_time_ns = res.exec_time_ns
        kernel._last_results = res
    return out


# revision 15
# speedup vs baseline: 1.2051x; 1.2051x over previous
"""ANFIS Trainium2 kernel (8 NeuronCores, Bass/Tile).

Math (reference):
  mfs[b,i,j] = exp(-(x[b,i]-centers[i,j])^2 / (2*widths[i,j]^2))   [1024,8,4]
  w[b,r]     = prod_i mfs[b,i,idx_i(r)]    r in [0, 4^8=65536), i0 slowest
  w        <- w / sum_r w
  out[b,n]   = sum_r w[b,r] * ([x[b],1] . rule_params[r,:,n])      [1024,16]

Key structure: w = wA (x) wB with wA over dims 0..2 (64 vals), wB over dims
3..7 (1024 vals); r = rA*1024 + rB.  The denominator factorizes:
sum_r w = prod_i (sum_j mfs[b,i,j]).

Sharding: rA split across 8 cores (8 local rA each = contiguous 8192-rule
row blocks of rule_params).  Per core:
  out_c[b,n] = sum_{rA local} wA[b,rA]/denom[b] *
               sum_i xb[b,i] (sum_rB wB[b,rB] rp[rA*1024+rB, i*16+n])
then a 2-way-split ReduceScatter(add) over cores; each core emits its two
64-row output shards, host reassembles.

The inner sum_rB is a matmul (contraction over rB on partitions), so wB^T
is needed in [rB, b] layout.  wB is built in [b, rB] layout with strided
free-axis outer products over dims 3..6 (w3456, 256 wide per b-tile),
scaled by the dim-7 membership (4 variants), and transposed 128x128 via
the DMA xbar transpose engine (bf16) -- no TensorE/PSUM involvement.
rB is enumerated as rB' = i7*256 + q (q = dims 3..6); rule_params rows are
permuted on the host to match.  Matmul operands are bf16 (fp32 matmuls
cost two PE passes); PSUM accumulation stays fp32.
"""

import sys

sys.path.insert(0, "/opt/trn_rl_repo")

import numpy as np

import concourse.bass as bass
import concourse.bacc as bacc
import concourse.tile as tile
import concourse.mybir as mybir
from concourse.ap import AP
from concourse.bass_utils import run_bass_kernel_spmd

F32 = mybir.dt.float32
BF16 = mybir.dt.bfloat16
MULT = mybir.AluOpType.mult
ADD = mybir.AluOpType.add
SUB = mybir.AluOpType.subtract
EXP = mybir.ActivationFunctionType.Exp
AXX = mybir.AxisListType.X

N_CORES = 8
B = 1024
BT = 8          # batch tiles of 128
D = 8           # input dims
DX = D + 1      # xb width (x plus ones column)
M = 4           # membership fns per dim
NO = 16         # outputs
C = DX * NO                 # 144
NRA = 64        # 4^3 (dims 0..2)
RA_LOC = NRA // N_CORES     # 8 local rA per core
NRB = 1024      # 4^5 (dims 3..7)
KT = 8          # rB partition tiles of 128
# rA groups per psum bank (N<=512): {0,1,2},{3,4,5},{6,7}
GROUPS = [(0, 3), (3, 3), (6, 2)]
SC = RA_LOC * C  # 1152


def _v(t, off, dims):
    """Custom free-dim view of a [128, F] SBUF tile AP.

    t: AP covering the full tile ([128, F]); off: element offset within the
    partition row; dims: list of (step, count) free dims, outer..inner.
    """
    part = list(t.ap[0])
    return AP(
        tensor=t.tensor,
        offset=t.offset + off,
        ap=[part] + [[s, n] for (s, n) in dims],
    )


def build_nc():
    nc = bacc.Bacc("TRN2", target_bir_lowering=False, debug=False,
                   num_devices=N_CORES)

    NSM = BT * DX + 2 * D * M + 2 * RA_LOC * 3  # 184
    small_d = nc.declare_dram_parameter("small", [128, NSM], F32, isOutput=False)
    eye_d = nc.declare_dram_parameter("eye", [128, 128], BF16, isOutput=False)
    rp_d = nc.declare_dram_parameter("rp", [128, KT * SC], BF16, isOutput=False)
    out_d = nc.declare_dram_parameter("out", [128, NO], F32, isOutput=True)

    with tile.TileContext(nc) as tc:
        with (
            tc.tile_pool(name="const", bufs=1) as cpool,
            tc.tile_pool(name="rp", bufs=1) as rppool,
            tc.tile_pool(name="wbt", bufs=1) as wbtpool,
            tc.tile_pool(name="work", bufs=2) as work,
            tc.tile_pool(name="dj", bufs=6) as djpool,
            tc.tile_pool(name="psD", bufs=1, space="PSUM") as psDp,
            tc.tile_pool(name="evac", bufs=2) as evpool,
            tc.tile_pool(name="ps0", bufs=2, space="PSUM") as ps0p,
            tc.tile_pool(name="ps1", bufs=2, space="PSUM") as ps1p,
            tc.tile_pool(name="ps2", bufs=2, space="PSUM") as ps2p,
            tc.tile_pool(name="dram", bufs=1, space="DRAM") as dram,
        ):
            # ---- input DMAs (one small slab + eye + rp slabs) ----
            small = cpool.tile([128, NSM], F32, tag="small")
            eye = cpool.tile([128, 128], BF16, tag="eye")
            rp = rppool.tile([128, KT * SC], BF16, tag="rp")

            nc.sync.dma_start(small[:], small_d[:])
            nc.sync.dma_start(eye[:], eye_d[:])
            xab = small[:, 0: BT * DX]
            cb = small[:, BT * DX: BT * DX + D * M]
            wt = small[:, BT * DX + D * M: BT * DX + 2 * D * M]
            cA = small[:, BT * DX + 2 * D * M: BT * DX + 2 * D * M + RA_LOC * 3]
            wtA = small[:, BT * DX + 2 * D * M + RA_LOC * 3: NSM]
            for kt in range(KT):
                nc.sync.dma_start(rp[:, kt * SC:(kt + 1) * SC],
                                  rp_d[:, kt * SC:(kt + 1) * SC])

            # ---- membership values mfs [128, (bt, i, j)] ----
            t32a = work.tile([128, D * M], F32, tag="t32")
            t32b = work.tile([128, D * M], F32, tag="t32")
            nw = cpool.tile([128, D * M], F32, tag="nw")
            nc.vector.tensor_tensor(t32a[:], wt, wt, op=MULT)
            nc.vector.tensor_scalar_mul(t32b[:], t32a[:], -2.0)
            nc.vector.reciprocal(nw[:], t32b[:])

            MF = BT * D * M  # 256
            dif = work.tile([128, MF], F32, tag="dif")
            nc.vector.tensor_tensor(
                dif[:],
                _v(xab, 0, [(DX, BT), (1, D), (0, M)]),
                _v(cb, 0, [(0, BT), (1, D * M)]),
                op=SUB,
            )
            d2 = work.tile([128, MF], F32, tag="d2")
            nc.vector.tensor_tensor(d2[:], dif[:], dif[:], op=MULT)
            d2s = work.tile([128, MF], F32, tag="d2s")
            nc.vector.tensor_tensor(
                d2s[:], d2[:], _v(nw[:], 0, [(0, BT), (1, D * M)]), op=MULT)
            mfs = cpool.tile([128, MF], F32, tag="mfs")
            nc.scalar.activation(mfs[:], d2s[:], EXP)

            # ---- wB factors over dims 3..6, [b, (bt, q)] layout, bf16 ----
            w34 = work.tile([128, BT * 16], BF16, tag="w34")
            nc.vector.tensor_tensor(
                w34[:],
                _v(mfs[:], 3 * M, [(D * M, BT), (1, M), (0, M)]),
                _v(mfs[:], 4 * M, [(D * M, BT), (0, M), (1, M)]),
                op=MULT)
            w56 = work.tile([128, BT * 16], BF16, tag="w56")
            nc.vector.tensor_tensor(
                w56[:],
                _v(mfs[:], 5 * M, [(D * M, BT), (1, M), (0, M)]),
                _v(mfs[:], 6 * M, [(D * M, BT), (0, M), (1, M)]),
                op=MULT)
            w3456 = cpool.tile([128, BT * 256], BF16, tag="w3456")
            i_w3456 = nc.vector.tensor_tensor(
                w3456[:],
                _v(w34[:], 0, [(16, BT), (1, 16), (0, 16)]),
                _v(w56[:], 0, [(16, BT), (0, 16), (1, 16)]),
                op=MULT)

            # ---- wB^T: pre-scale by dim-7 membership, PE-transpose vs eye ----
            # wbt[p, kt*1024 + b] = wB'[rB' = kt*128+p, b],  rB' = i7*256+q
            # w3s = w3456 * mfs7_j;  psum = w3s.T @ I
            wbt = wbtpool.tile([128, KT * B], BF16, tag="wbt")
            for bt in range(BT):
                w3s = []
                for j in range(M):
                    w3sj = djpool.tile([128, 256], BF16, tag="w3s", name="w3s")
                    nc.vector.tensor_scalar_mul(
                        w3sj[:], w3456[:, bt * 256:(bt + 1) * 256],
                        mfs[:, bt * D * M + 7 * M + j:
                            bt * D * M + 7 * M + j + 1])
                    w3s.append(w3sj)
                psD = [psDp.tile([128, 512], F32, tag="psD0", name="psD0"),
                       psDp.tile([128, 512], F32, tag="psD1", name="psD1")]
                for j in range(M):
                    for qh in range(2):
                        kt = 2 * j + qh
                        m, t = kt // 4, kt % 4
                        nc.tensor.matmul(
                            psD[m][:, t * 128:(t + 1) * 128],
                            w3s[j][:, qh * 128:(qh + 1) * 128], eye[:],
                            start=True, stop=True)
                for m in range(2):
                    nc.scalar.copy(
                        _v(wbt[:], (4 * m) * B + bt * 128, [(B, 4), (1, 128)]),
                        psD[m][:])

            # ---- local wA [128, (bt, r)] from per-core selected centers ----
            t24a = work.tile([128, RA_LOC * 3], F32, tag="t24")
            t24b = work.tile([128, RA_LOC * 3], F32, tag="t24")
            nwA = cpool.tile([128, RA_LOC * 3], F32, tag="nwA")
            i_t24a = nc.vector.tensor_tensor(t24a[:], wtA, wtA, op=MULT)
            tile.add_dep_helper(i_t24a.ins, i_w3456.ins, sync=False,
                                reason="keep wbt build chain ahead of wA chain")
            nc.vector.tensor_scalar_mul(t24b[:], t24a[:], -2.0)
            nc.vector.reciprocal(nwA[:], t24b[:])

            NA = BT * RA_LOC * 3  # 192
            dA = work.tile([128, NA], F32, tag="dA")
            nc.vector.tensor_tensor(
                dA[:],
                _v(xab, 0, [(DX, BT), (0, RA_LOC), (1, 3)]),
                _v(cA, 0, [(0, BT), (3, RA_LOC), (1, 3)]),
                op=SUB,
            )
            dA2 = work.tile([128, NA], F32, tag="dA2")
            nc.vector.tensor_tensor(dA2[:], dA[:], dA[:], op=MULT)
            dA2s = work.tile([128, NA], F32, tag="dA2s")
            nc.vector.tensor_tensor(
                dA2s[:], dA2[:],
                _v(nwA[:], 0, [(0, BT), (3, RA_LOC), (1, 3)]), op=MULT)
            eA = work.tile([128, BT * RA_LOC], F32, tag="eA")
            nc.vector.reduce_sum(
                eA[:], _v(dA2s[:], 0, [(3 * RA_LOC, BT), (3, RA_LOC), (1, 3)]),
                axis=AXX)
            wA = cpool.tile([128, BT * RA_LOC], F32, tag="wA")
            nc.scalar.activation(wA[:], eA[:], EXP)

            # ---- denominator: denom[b] = prod_i sum_j mfs ----
            s = work.tile([128, BT * D], F32, tag="s")
            nc.vector.reduce_sum(
                s[:], _v(mfs[:], 0, [(M, BT * D), (1, M)]), axis=AXX)
            p1 = work.tile([128, BT * 4], F32, tag="p1")
            nc.vector.tensor_tensor(
                p1[:], _v(s[:], 0, [(D, BT), (1, 4)]),
                _v(s[:], 4, [(D, BT), (1, 4)]), op=MULT)
            p2 = work.tile([128, BT * 2], F32, tag="p2")
            nc.vector.tensor_tensor(
                p2[:], _v(p1[:], 0, [(4, BT), (1, 2)]),
                _v(p1[:], 2, [(4, BT), (1, 2)]), op=MULT)
            p3 = work.tile([128, BT], F32, tag="p3")
            nc.vector.tensor_tensor(
                p3[:], _v(p2[:], 0, [(2, BT)]), _v(p2[:], 1, [(2, BT)]),
                op=MULT)
            invd = cpool.tile([128, BT], F32, tag="invd")
            nc.vector.reciprocal(invd[:], p3[:])

            wAn = cpool.tile([128, BT * RA_LOC], F32, tag="wAn")
            for bt in range(BT):
                nc.vector.tensor_scalar_mul(
                    wAn[:, bt * RA_LOC:(bt + 1) * RA_LOC],
                    wA[:, bt * RA_LOC:(bt + 1) * RA_LOC],
                    invd[:, bt:bt + 1])

            # ---- main matmuls + evac ----
            partial1 = dram.tile([B // 2, NO], F32)
            partial2 = dram.tile([B // 2, NO], F32)
            for bt in range(BT):
                ps = [ps0p.tile([128, GROUPS[0][1] * C], F32, tag="ps0", name="ps0"),
                      ps1p.tile([128, GROUPS[1][1] * C], F32, tag="ps1", name="ps1"),
                      ps2p.tile([128, GROUPS[2][1] * C], F32, tag="ps2", name="ps2")]
                for kt in range(KT):
                    lhsT = wbt[:, kt * B + bt * 128: kt * B + (bt + 1) * 128]
                    for g, (r0, nr) in enumerate(GROUPS):
                        nc.tensor.matmul(
                            ps[g][:], lhsT,
                            _v(rp[:], (kt * RA_LOC + r0) * C, [(C, nr), (1, C)]),
                            start=(kt == 0), stop=(kt == KT - 1))
                # xsc[b, r*144 + i*16 + n] = psum * wAn[b,r] * xb[b,i]
                # via G[b, r*9+i] = wAn[b,r]*xb[b,i], then one big reduce
                G = evpool.tile([128, RA_LOC * DX], F32, tag="G")
                nc.vector.tensor_tensor(
                    G[:],
                    _v(wAn[:], bt * RA_LOC, [(1, RA_LOC), (0, DX)]),
                    _v(xab, bt * DX, [(0, RA_LOC), (1, DX)]), op=MULT)
                xsc = evpool.tile([128, SC], F32, tag="xsc")
                for g, (r0, nr) in enumerate(GROUPS):
                    nc.vector.tensor_tensor(
                        xsc[:, r0 * C:(r0 + nr) * C], ps[g][:],
                        _v(G[:], r0 * DX, [(DX, nr), (1, DX), (0, NO)]),
                        op=MULT)
                ob = evpool.tile([128, NO], F32, tag="ob")
                nc.vector.reduce_sum(
                    ob[:], _v(xsc[:], 0, [(1, NO), (NO, RA_LOC * DX)]),
                    axis=AXX)
                pdst = partial1 if bt < BT // 2 else partial2
                row0 = (bt % (BT // 2)) * 128
                nc.sync.dma_start(pdst[row0:row0 + 128, :], ob[:])

            # ---- split reduce-scatter; each core keeps two 64-row shards ----
            rs1 = dram.tile([B // N_CORES // 2, NO], F32)
            rs2 = dram.tile([B // N_CORES // 2, NO], F32)
            nc.gpsimd.collective_compute(
                "ReduceScatter", ADD,
                replica_groups=[list(range(N_CORES))],
                ins=[partial1.opt()], outs=[rs1.opt()])
            nc.gpsimd.collective_compute(
                "ReduceScatter", ADD,
                replica_groups=[list(range(N_CORES))],
                ins=[partial2.opt()], outs=[rs2.opt()])
            nc.sync.dma_start(out_d[0:64, :], rs1[:])
            nc.sync.dma_start(out_d[64:128, :], rs2[:])

    nc.compile()
    return nc


_NC_CACHE = None


def _get_nc():
    global _NC_CACHE
    if _NC_CACHE is None:
        _NC_CACHE = build_nc()
    return _NC_CACHE


def _prep_in_maps(x, centers, widths, rule_params):
    import ml_dtypes

    x = np.asarray(x, np.float32)
    centers = np.asarray(centers, np.float32)
    widths = np.asarray(widths, np.float32)
    rule_params = np.asarray(rule_params, np.float32)

    # xab[p, bt*9+i] = x[bt*128+p, i] for i<8; 1.0 at i=8
    xab = np.ones((128, BT, DX), np.float32)
    xab[:, :, :D] = x.reshape(BT, 128, D).transpose(1, 0, 2)
    xab = xab.reshape(128, BT * DX)
    cb = np.broadcast_to(centers.reshape(1, D * M), (128, D * M))
    wt = np.broadcast_to(widths.reshape(1, D * M), (128, D * M))
    eye = np.eye(128, dtype=ml_dtypes.bfloat16)

    # rule_params rows r = rA*1024 + q*4 + j -> per core [p, kt, rA, c]
    # with row order rB' = j*256 + q, kt = rB' tile of 128.
    rp4 = rule_params.reshape(NRA, 256, M, C).transpose(0, 2, 1, 3)
    rp4 = rp4.reshape(NRA, NRB, C)

    in_maps = []
    for c in range(N_CORES):
        ra0 = c * RA_LOC
        idx = np.empty((RA_LOC, 3), np.int64)
        for r in range(RA_LOC):
            ra = ra0 + r
            idx[r] = [(ra >> 4) & 3, (ra >> 2) & 3, ra & 3]
        k = np.arange(3)
        cA = centers[k[None, :], idx]
        wtA = widths[k[None, :], idx]
        cA = np.broadcast_to(cA.reshape(1, RA_LOC * 3), (128, RA_LOC * 3))
        wtA = np.broadcast_to(wtA.reshape(1, RA_LOC * 3), (128, RA_LOC * 3))
        small = np.ascontiguousarray(
            np.concatenate([xab, cb, wt, cA, wtA], axis=1, dtype=np.float32))

        rp_c = rp4[ra0:ra0 + RA_LOC]                     # [8, 1024, 144]
        rp_c = rp_c.reshape(RA_LOC, KT, 128, C).transpose(2, 1, 0, 3)
        rp_c = np.ascontiguousarray(
            rp_c.reshape(128, KT * SC)).astype(ml_dtypes.bfloat16)

        in_maps.append({"small": small, "eye": eye, "rp": rp_c})
    return in_maps


def kernel(x, centers, widths, rule_params, _trace=False):
    nc = _get_nc()
    in_maps = _prep_in_maps(x, centers, widths, rule_params)
    res = run_bass_kernel_spmd(nc, in_maps, core_ids=list(range(N_CORES)),
                               trace=_trace)
    out = np.empty((B, NO), np.float32)
    for c in range(N_CORES):
        oc = res.results[c]["out"]
        out[c * 64:(c + 1) * 64] = oc[0:64]
        out[B // 2 + c * 64: B // 2 + (c + 1) * 64] = oc[64:128]
    if _trace:
        kernel._last_exec_time_ns = res.exec_time_ns
        kernel._last_results = res
    return out
